# revision 5
# baseline (speedup 1.0000x reference)
"""BatchChildSumTreeLSTM Trainium2 kernel, v2.

Forest of T complete B-ary trees, processed leaves-first, 2 trees per
core.  Differences vs v1:

- Sibling-major node order within each level (host permutes embeds):
  children of the parent at position p are at positions {s*n_par + p},
  i.e. 4 contiguous blocks per level.  All child-sum reductions become
  contiguous block adds (DVE 2x_1p eligible) and the f-gate's parent
  embedding is the parent level's own xT slice, so the duplicated `xb`
  stream is gone entirely (half the DMA traffic).
- Every elementwise tensor is bf16 (including the c chain), putting all
  VectorE tensor_tensor ops into the 2x_1p perf mode.
- Leaf tanh(c): c_leaf = i*u is strictly inside (-1,1), so tanh is
  evaluated as a degree-3 odd minimax polynomial on VectorE
  (tensor_scalar runs at 4x) instead of on the saturated ScalarE.
- Parent (level-6) gate groups are deferred one full leaf iteration so
  the h_child_sum -> matmul -> sigmoid recurrence is never on ScalarE's
  critical path.
"""

import sys

if "/opt/trn_rl_repo" not in sys.path:
    sys.path.insert(0, "/opt/trn_rl_repo")

import numpy as np

P = 128          # feature dim == partitions
BR = 4           # branching factor
NLBL = 5

# degree-3 odd minimax fit of tanh on [-1, 1], max err 6.9e-3
A1, A3 = 0.9668137, -0.20713426

_NC_CACHE = {}


def _levels(tpc, depth):
    n = [tpc * BR**l for l in range(depth)]
    off = [0]
    for c in n:
        off.append(off[-1] + c)
    return n, off, off[-1]


def _legalize_waits(nc, max_waits=1):
    """This walrus build accepts at most one sync-wait command per
    instruction (any type).  Hoist excess waits onto same-engine NoOps
    inserted right before the instruction; engine program order makes
    this exactly equivalent."""
    import concourse.mybir as mybir

    n_nops = 0
    for fn in nc.m.functions:
        for blk in fn.blocks:
            new_insts = []
            for inst in blk.instructions:
                si = getattr(inst, "sync_info", None)
                if si is not None and si.on_wait and len(si.on_wait) > max_waits:
                    waits = list(si.on_wait)
                    hoist, keep = waits[:-max_waits], waits[-max_waits:]
                    eng = getattr(inst, "engine", None)
                    for j, w in enumerate(hoist):
                        nop = mybir.InstNoOp(
                            name=f"{inst.name}-wn{j}",
                            engine=eng,
                            bass_nofuse=True,
                        )
                        nop.sync_info = mybir.SyncInfo(on_wait=[w],
                                                       on_update=[])
                        new_insts.append(nop)
                        n_nops += 1
                    inst.sync_info = mybir.SyncInfo(
                        on_wait=keep, on_update=list(si.on_update))
                new_insts.append(inst)
            blk.instructions = new_insts
    return n_nops


def build_nc(tpc=2, depth=8, nlbl=NLBL, legalize=True):
    """Build the per-core Bass/Tile program (identical on all cores)."""
    import contextlib

    import concourse.bass as bass
    import concourse.mybir as mybir
    import concourse.tile as tile

    f32 = mybir.dt.float32
    bf16 = mybir.dt.bfloat16
    AF = mybir.ActivationFunctionType
    ADD = mybir.AluOpType.add
    MULT = mybir.AluOpType.mult

    n, off, ntot = _levels(tpc, depth)
    lleaf = depth - 1
    l6 = depth - 2
    G = 2048                 # parent group / leaf slice width
    NIT = n[l6] // G         # leaf+l6 iterations (4)

    nc = bass.Bass()

    xT = nc.dram_tensor("xT", [P, ntot], bf16, kind="ExternalInput")
    wnames = ["ix", "ux", "ox", "fx", "ih", "uh", "oh", "fh"]
    wall = nc.dram_tensor("Wall", [P, len(wnames), P], bf16,
                          kind="ExternalInput")
    bias4 = nc.dram_tensor("bias4", [P, 4], f32, kind="ExternalInput")
    woutT = nc.dram_tensor("WoutT", [P, nlbl], bf16, kind="ExternalInput")
    bout = nc.dram_tensor("bout2", [tpc, nlbl], f32, kind="ExternalInput")
    out = nc.dram_tensor("out", [tpc, nlbl], f32, kind="ExternalOutput")

    SIG, TANH = AF.Sigmoid, AF.Tanh
    BIDX = {"i": 0, "f": 1, "o": 2, "u": 3}

    with tile.TileContext(nc) as tc:
        with contextlib.ExitStack() as ctx:
            wp = ctx.enter_context(tc.tile_pool(name="wp", bufs=1))
            stream = ctx.enter_context(tc.tile_pool(name="stream", bufs=2))
            gt = ctx.enter_context(tc.tile_pool(name="gt", bufs=2))
            pl = ctx.enter_context(tc.tile_pool(name="pl", bufs=2))
            acc = ctx.enter_context(tc.tile_pool(name="acc", bufs=2))
            hcp = ctx.enter_context(tc.tile_pool(name="hcp", bufs=1))
            psum = ctx.enter_context(
                tc.tile_pool(name="psum", bufs=2, space="PSUM"))

            # ---- first leaf slice + constants ----
            # slice (0,0) first, in 512-col quarters, so the very first
            # gate matmul group can start on quarter 0 while the rest of
            # the boot DMAs stream in behind it
            xt7_first = stream.tile([P, G], bf16, name="xt7", tag="xt7",
                                    bufs=3)
            for q in range(4):
                nc.sync.dma_start(out=xt7_first[:, q * 512:(q + 1) * 512],
                                  in_=xT[:, off[lleaf] + q * 512:
                                         off[lleaf] + (q + 1) * 512])
            wall_sb = wp.tile([P, len(wnames), P], bf16, name="wall_sb",
                              tag="wall_sb")
            nc.sync.dma_start(out=wall_sb[:, 0:4, :], in_=wall[:, 0:4, :])
            wsb = {nm: wall_sb[:, j, :] for j, nm in enumerate(wnames)}
            bias_sb = wp.tile([P, 4], f32, name="bias_sb", tag="bias_sb")
            nc.sync.dma_start(out=bias_sb, in_=bias4[:])
            woutT_sb = wp.tile([P, nlbl], bf16, name="woutT_sb",
                               tag="woutT_sb")
            bout_sb = wp.tile([tpc, nlbl], f32, name="bout_sb", tag="bout_sb")

            def bias_ap(g):
                return bias_sb[:, BIDX[g]:BIDX[g] + 1]

            # persistent per-level h/c (levels 0..6), all bf16
            hres, cres = {}, {}
            for l in range(depth - 1):
                hres[l] = hcp.tile([P, n[l]], bf16, name=f"h{l}_sb",
                                   tag=f"h{l}_sb")
                cres[l] = hcp.tile([P, n[l]], bf16, name=f"c{l}_sb",
                                   tag=f"c{l}_sb")

            def matmul_group(ps, w0, rhs0, w1=None, rhs1=None, S=G):
                """ps[:, :S] = w0.T@rhs0 (+ w1.T@rhs1), 512-col banks."""
                nb = (S + 511) // 512
                for b in range(nb):
                    s = b * 512
                    e = min(s + 512, S)
                    nc.tensor.matmul(ps[:, s:e], wsb[w0], rhs0[:, s:e],
                                     start=True, stop=(w1 is None))
                if w1 is not None:
                    for b in range(nb):
                        s = b * 512
                        e = min(s + 512, S)
                        nc.tensor.matmul(ps[:, s:e], wsb[w1], rhs1[:, s:e],
                                         start=False, stop=True)

            def load_xt(a, b, tag="xtu", bufs=2):
                t = stream.tile([P, b - a], bf16, name=tag, tag=tag,
                                bufs=bufs)
                nc.sync.dma_start(out=t, in_=xT[:, a:b])
                return t

            def poly_tanh(tt_out, c_in, S):
                """tt_out = tanh(c_in) for |c_in| < 1, deg-3 odd poly on DVE."""
                s2 = pl.tile([P, S], bf16, name="s2", tag="s2", bufs=1)
                nc.vector.tensor_mul(s2, c_in, c_in)
                t1 = pl.tile([P, S], bf16, name="t1", tag="tq", bufs=1)
                nc.vector.tensor_scalar(t1, s2, A3, A1, MULT, ADD)
                nc.vector.tensor_mul(tt_out, t1, c_in)

            def gate_pass(xt_ap, hs_ap, fcs_ap, h_out, c_out, S,
                          leaf_poly=False):
                """i,u,o gates + c,h for S parent columns."""
                g_sb = {}
                for gname, wx, wh, func in (("i", "ix", "ih", SIG),
                                            ("u", "ux", "uh", TANH),
                                            ("o", "ox", "oh", SIG)):
                    ps = psum.tile([P, G], f32, name=f"ps_{gname}", tag="ps")
                    if hs_ap is None:
                        matmul_group(ps, wx, xt_ap, S=S)
                    else:
                        matmul_group(ps, wx, xt_ap, wh, hs_ap, S=S)
                    g = gt.tile([P, S], bf16, name=f"g_{gname}",
                                tag=f"g_{gname}")
                    nc.scalar.activation(out=g, in_=ps[:, :S], func=func,
                                         bias=bias_ap(gname), scale=1.0)
                    g_sb[gname] = g
                nc.vector.tensor_mul(c_out, g_sb["i"], g_sb["u"])
                if fcs_ap is not None:
                    nc.vector.tensor_add(c_out, c_out, fcs_ap)
                tt = gt.tile([P, S], bf16, name="tt", tag="tt", bufs=1)
                if leaf_poly:
                    poly_tanh(tt, c_out, S)
                else:
                    nc.scalar.activation(out=tt, in_=c_out, func=TANH,
                                         bias=0.0, scale=1.0)
                nc.vector.tensor_mul(h_out, g_sb["o"], tt)

            def f_sigmoid(xt_par_ap, h_ch_ap, c_ch_ap, fc_out, S):
                """fc_out = sigmoid(Wfx x_par + Wfh h_ch + bfx) * c_ch for one
                sibling block of S columns (x_par cols == child block cols)."""
                ps = psum.tile([P, G], f32, name="ps_f", tag="ps")
                matmul_group(ps, "fx", xt_par_ap, "fh", h_ch_ap, S=S)
                f = gt.tile([P, S], bf16, name="g_f", tag="g_f")
                nc.scalar.activation(out=f, in_=ps[:, :S], func=SIG,
                                     bias=bias_ap("f"), scale=1.0)
                nc.vector.tensor_mul(fc_out, f, c_ch_ap)

            # all upper-level (0..5) embeddings in one resident tile
            # (DMA issued after the first leaf prefetches, see below)
            nup = off[depth - 2]
            xtu_all = wp.tile([P, nup], bf16, name="xtu_all", tag="xtu_all")

            def xt_of(l):
                return xtu_all[:, off[l]:off[l + 1]]

            l5 = depth - 3
            np5 = n[l5]          # 2048
            l5st = {"fc": [], "fp1": None, "hp1": None}

            def emit_l5_fblock(s5):
                """f-pass of level-6 sibling block s5 toward level 5,
                pipelined into the leaf iterations."""
                h6b = hres[l6][:, s5 * np5:(s5 + 1) * np5]
                c6b = cres[l6][:, s5 * np5:(s5 + 1) * np5]
                fct = acc.tile([P, np5], bf16, name="fc5", tag="fc5", bufs=2)
                f_sigmoid(xt_of(l5), h6b, c6b, fct, np5)
                l5st["fc"].append(fct)
                if s5 == 1:
                    l5st["fp1"] = acc.tile([P, np5], bf16, name="fp1",
                                           tag="fr5", bufs=2)
                    nc.vector.tensor_add(l5st["fp1"], l5st["fc"][0],
                                         l5st["fc"][1])
                    l5st["hp1"] = acc.tile([P, np5], bf16, name="hp1",
                                           tag="hr5", bufs=2)
                    nc.vector.tensor_add(l5st["hp1"],
                                         hres[l6][:, 0:np5],
                                         hres[l6][:, np5:2 * np5])

            # ================= leaves + level 6, 4 iterations =============
            # Software pipeline over 16 leaf slices (4 iterations x 4
            # sibling blocks).  Each slice's f-pass is deferred one slice
            # so the in-order PE never waits on the slice's own DVE h
            # chain; each l6 parent-gate group is deferred one further
            # slice past its last fc reduction.
            nl6 = n[l6]          # 8192
            pend_gates = None    # (a6, xt6, hs6, fcs6)
            pend_f = None        # (it, s, xt6, h7s, c7s)
            st = {}              # per-iteration reduction state

            def l6_part(p, part):
                """One third of a deferred l6 gate group: i at the first
                step, u+c at the second, o+tanh+h at the third -- keeps the
                PE burst per leaf step small so ScalarE never drains."""
                a6, xt6p, hs6p, fcs6p, g6 = p
                c6s = cres[l6][:, a6:a6 + G]
                h6s = hres[l6][:, a6:a6 + G]
                gname, wx, wh, func = (("i", "ix", "ih", SIG),
                                       ("u", "ux", "uh", TANH),
                                       ("o", "ox", "oh", SIG))[part]
                ps = psum.tile([P, G], f32, name=f"ps_l6{gname}", tag="ps")
                matmul_group(ps, wx, xt6p, wh, hs6p, S=G)
                g = gt.tile([P, G], bf16, name=f"g6{gname}",
                            tag=f"g_{gname}")
                nc.scalar.activation(out=g, in_=ps[:, :G], func=func,
                                     bias=bias_ap(gname), scale=1.0)
                g6[gname] = g
                if part == 1:
                    nc.vector.tensor_mul(c6s, g6["i"], g6["u"])
                    nc.vector.tensor_add(c6s, c6s, fcs6p)
                if part == 2:
                    tt = gt.tile([P, G], bf16, name="tt6", tag="tt6",
                                 bufs=1)
                    nc.scalar.activation(out=tt, in_=c6s, func=TANH,
                                         bias=0.0, scale=1.0)
                    nc.vector.tensor_mul(h6s, g6["o"], tt)

            def emit_l6_gates(p):
                for part in range(3):
                    l6_part(p, part)

            def emit_f(p):
                """Deferred f-pass of one leaf slice + fc reductions."""
                it, s, xt6p, h7s, c7s = p
                fc_t = acc.tile([P, G], bf16, name="fc", tag="fc", bufs=3)
                f_sigmoid(xt6p, h7s, c7s, fc_t, G)
                r = st[it]
                r["fc"].append(fc_t)
                if s == 1:
                    r["fr01"] = acc.tile([P, G], bf16, name="fr01",
                                         tag="fr", bufs=2)
                    nc.vector.tensor_add(r["fr01"], r["fc"][0], r["fc"][1])
                if s == 3:
                    fr23 = acc.tile([P, G], bf16, name="fr23", tag="fr",
                                    bufs=2)
                    nc.vector.tensor_add(fr23, r["fc"][2], r["fc"][3])
                    fcs6 = acc.tile([P, G], bf16, name="fcs6", tag="fcs6",
                                    bufs=1)
                    nc.vector.tensor_add(fcs6, r["fr01"], fr23)
                    r["fcs6"] = fcs6

            # prefetch iteration 0 inputs
            xt7_next = [xt7_first] + [
                load_xt(off[lleaf] + s * nl6, off[lleaf] + s * nl6 + G,
                        tag="xt7", bufs=3) for s in range(1, BR)]
            xt6_next = load_xt(off[l6], off[l6] + G, tag="xt6", bufs=2)
            nc.sync.dma_start(out=wall_sb[:, 4:8, :], in_=wall[:, 4:8, :])
            nc.sync.dma_start(out=woutT_sb, in_=woutT[:])
            nc.sync.dma_start(out=bout_sb, in_=bout[:])
            nc.sync.dma_start(out=xtu_all, in_=xT[:, 0:nup])

            for it in range(NIT):
                a6 = it * G
                xt7s, xt6c = xt7_next, xt6_next
                st[it] = {"fc": [], "h7": []}
                if it + 1 < NIT:
                    b6 = (it + 1) * G
                    xt7_next = [load_xt(off[lleaf] + s * nl6 + b6,
                                        off[lleaf] + s * nl6 + b6 + G,
                                        tag="xt7", bufs=3) for s in range(BR)]
                    xt6_next = load_xt(off[l6] + b6, off[l6] + b6 + G,
                                       tag="xt6", bufs=2)
                for s in range(BR):
                    last = (it == NIT - 1)
                    c7s = pl.tile([P, G], bf16, name="c7", tag="c7", bufs=2)
                    h7s = pl.tile([P, G], bf16, name="h7", tag="h7", bufs=3)
                    gate_pass(xt7s[s], None, None, h7s, c7s, G,
                              leaf_poly=not (last and s == 3))
                    r = st[it]
                    r["h7"].append(h7s)
                    if s == 1:
                        r["hr01"] = acc.tile([P, G], bf16, name="hr01",
                                             tag="hr", bufs=2)
                        nc.vector.tensor_add(r["hr01"], r["h7"][0],
                                             r["h7"][1])
                    if s == 3:
                        hr23 = acc.tile([P, G], bf16, name="hr23", tag="hr",
                                        bufs=2)
                        nc.vector.tensor_add(hr23, r["h7"][2], r["h7"][3])
                        hs6 = acc.tile([P, G], bf16, name="hs6", tag="hs6",
                                       bufs=2)
                        nc.vector.tensor_add(hs6, r["hr01"], hr23)
                        r["hs6"] = hs6
                    if pend_f is not None:
                        emit_f(pend_f)
                        pend_f = None
                    if last and s == 3:
                        emit_f((it, s, xt6c, h7s, c7s))
                    else:
                        pend_f = (it, s, xt6c, h7s, c7s)
                    if s >= 1 and it > 0:
                        if s == 1:
                            p = st.pop(it - 1)
                            st["g"] = ((it - 1) * G, p["xt6"], p["hs6"],
                                       p["fcs6"], {})
                        l6_part(st["g"], s - 1)
                    if s == 3 and it > 0:
                        emit_l5_fblock(it - 1)
                st[it]["xt6"] = xt6c
            # drain: last l6 group (its f-pass was not deferred)
            p = st.pop(NIT - 1)
            emit_l6_gates(((NIT - 1) * G, p["xt6"], p["hs6"], p["fcs6"], {}))

            # ================= level 5 (drain) ============================
            emit_l5_fblock(3)
            fp2 = acc.tile([P, np5], bf16, name="fp2", tag="fr5", bufs=2)
            nc.vector.tensor_add(fp2, l5st["fc"][2], l5st["fc"][3])
            fcs5 = acc.tile([P, np5], bf16, name="fcs5", tag="fcs6", bufs=1)
            nc.vector.tensor_add(fcs5, l5st["fp1"], fp2)
            hp2 = acc.tile([P, np5], bf16, name="hp2", tag="hr5", bufs=2)
            nc.vector.tensor_add(hp2, hres[l6][:, 2 * np5:3 * np5],
                                 hres[l6][:, 3 * np5:])
            hs5 = acc.tile([P, np5], bf16, name="hs5", tag="hs6", bufs=2)
            nc.vector.tensor_add(hs5, l5st["hp1"], hp2)
            gate_pass(xt_of(l5), hs5, fcs5, hres[l5], cres[l5], np5)

            # ================= levels 4..0 ================================
            # Tail latency is chain-dominated, so the f-chain
            # (fmm->sigmoid->fc->reduce) and the gate chain
            # (hsum->hmm->sigmoid->c) are arranged to run concurrently:
            # gate x-matmuls open per-bank PSUM slots up front, the h child
            # sum (which needs only level l+1's h) feeds the gate h-matmuls
            # without waiting on anything f-related, and c+=fcs is the only
            # join point.
            GATES3 = (("i", "ix", "ih", SIG), ("u", "ux", "uh", TANH),
                      ("o", "ox", "oh", SIG))
            SL = 512
            for l in range(depth - 4, -1, -1):
                npar = n[l]
                nch = n[l + 1]           # 4*npar <= 2048
                xtl = xt_of(l)
                if l <= depth - 5:
                    # keep-warm: dependency-free matmuls execute during the
                    # wait on level l+1's h, so the PE clock does not drop
                    # to its cold p-state between the tiny tail levels
                    warm = psum.tile([P, G], f32, name="ps_warm", tag="ps")
                    for _ in range(2):
                        nc.tensor.matmul(warm[:, 0:512], wsb["ix"],
                                         xtu_all[:, 0:512],
                                         start=True, stop=True)
                # h child-sum first: depends only on level l+1 gates
                hr1 = acc.tile([P, 2 * npar], bf16, name="hr1", tag="hr",
                               bufs=2)
                nc.vector.tensor_add(hr1, hres[l + 1][:, :2 * npar],
                                     hres[l + 1][:, 2 * npar:])
                hsl = acc.tile([P, npar], bf16, name="hsl", tag="hs6",
                               bufs=2)
                nc.vector.tensor_add(hsl, hr1[:, :npar], hr1[:, npar:])
                # gate x-parts into three 1-bank PSUM slots, groups left open
                ps_g = psum.tile([P, G], f32, name="ps_g", tag="ps")
                for gi, (gname, wx, wh, func) in enumerate(GATES3):
                    nc.tensor.matmul(ps_g[:, gi * SL:gi * SL + npar],
                                     wsb[wx], xtl, start=True, stop=False)
                # f-pass (own tile when the child level fills 4 banks)
                if nch > SL:
                    ps_f = psum.tile([P, G], f32, name="ps_f", tag="ps")
                    fof = 0
                else:
                    ps_f, fof = ps_g, 3 * SL
                for s in range(BR):
                    sl_ = slice(fof + s * npar, fof + (s + 1) * npar)
                    nc.tensor.matmul(ps_f[:, sl_], wsb["fx"], xtl,
                                     start=True, stop=False)
                    nc.tensor.matmul(ps_f[:, sl_], wsb["fh"],
                                     hres[l + 1][:, s * npar:(s + 1) * npar],
                                     start=False, stop=True)
                # gate h-parts: need only hsl, run while ACT does sigmoid(f)
                for gi, (gname, wx, wh, func) in enumerate(GATES3):
                    nc.tensor.matmul(ps_g[:, gi * SL:gi * SL + npar],
                                     wsb[wh], hsl, start=False, stop=True)
                f = gt.tile([P, nch], bf16, name="g_f", tag="g_f")
                nc.scalar.activation(out=f, in_=ps_f[:, fof:fof + nch],
                                     func=SIG, bias=bias_ap("f"), scale=1.0)
                g_sb = {}
                for gi, (gname, wx, wh, func) in enumerate(GATES3):
                    g = gt.tile([P, npar], bf16, name=f"g_{gname}",
                                tag=f"g_{gname}")
                    nc.scalar.activation(out=g,
                                         in_=ps_g[:, gi * SL:gi * SL + npar],
                                         func=func, bias=bias_ap(gname),
                                         scale=1.0)
                    g_sb[gname] = g
                fct = acc.tile([P, nch], bf16, name="fcl", tag="fc", bufs=3)
                nc.vector.tensor_mul(fct, f, cres[l + 1])
                r1 = acc.tile([P, 2 * npar], bf16, name="r1", tag="fr",
                              bufs=2)
                nc.vector.tensor_add(r1, fct[:, :2 * npar],
                                     fct[:, 2 * npar:])
                fcsl = acc.tile([P, npar], bf16, name="fcsl", tag="fcs6",
                                bufs=1)
                nc.vector.tensor_add(fcsl, r1[:, :npar], r1[:, npar:])
                c_out = cres[l]
                h_out = hres[l]
                nc.vector.tensor_mul(c_out, g_sb["i"], g_sb["u"])
                nc.vector.tensor_add(c_out, c_out, fcsl)
                tt = gt.tile([P, npar], bf16, name="tt", tag="tt", bufs=1)
                nc.scalar.activation(out=tt, in_=c_out, func=TANH,
                                     bias=0.0, scale=1.0)
                nc.vector.tensor_mul(h_out, g_sb["o"], tt)

            # ================= output head ================================
            ps = psum.tile([P, G], f32, name="ps_out", tag="ps")
            nc.tensor.matmul(ps[:tpc, :nlbl], hres[0], woutT_sb,
                             start=True, stop=True)
            out_sb = gt.tile([tpc, nlbl], f32, name="out_sb", tag="out_sb")
            nc.vector.tensor_add(out_sb, ps[:tpc, :nlbl], bout_sb)
            nc.sync.dma_start(out=out[:], in_=out_sb)

    if legalize:
        _legalize_waits(nc)
    return nc


def _sibling_major_orig(tpc, depth):
    """orig[l][pos] = core-local original index of the node at position
    `pos` in the sibling-major order of level l."""
    orig = [np.arange(tpc, dtype=np.int64)]
    for _ in range(1, depth):
        prev = orig[-1]
        orig.append(np.concatenate([prev * BR + s for s in range(BR)]))
    return orig


def _prep_core_inputs(embeds, weights, tpc, depth, n_cores):
    """Host-side shard + sibling-major permute + transpose per core."""
    import ml_dtypes
    bf16 = ml_dtypes.bfloat16

    T = tpc * n_cores
    counts = [T * BR**l for l in range(depth)]
    offsets = [0]
    for c in counts:
        offsets.append(offsets[-1] + c)
    orig = _sibling_major_orig(tpc, depth)

    in_maps = []
    for d in range(n_cores):
        rows = np.concatenate(
            [offsets[l] + tpc * d * BR**l + orig[l] for l in range(depth)])
        shard = embeds[rows]
        xT = np.ascontiguousarray(shard.T.astype(bf16))   # [P, ntot]
        m = {"xT": xT}
        m.update(weights)
        in_maps.append(m)
    return in_maps


def _prep_weights(Wix, bix, Wih, Wfx, bfx, Wfh, Wox, box, Woh, Wux, bux, Wuh,
                  Wout, bout, tpc):
    import ml_dtypes
    f = np.float32
    bf = ml_dtypes.bfloat16
    # order must match build_nc's wnames: ix, ux, ox, fx, ih, uh, oh, fh
    wall = np.stack([Wix.T, Wux.T, Wox.T, Wfx.T, Wih.T, Wuh.T, Woh.T, Wfh.T],
                    axis=1)   # [128 (in-feat), 8, 128 (out-feat)]
    w = {
        "Wall": np.ascontiguousarray(wall, dtype=bf),
        "bias4": np.ascontiguousarray(
            np.stack([bix, bfx, box, bux], axis=1), dtype=f),
        "WoutT": np.ascontiguousarray(Wout.T, dtype=bf),
        "bout2": np.ascontiguousarray(np.tile(bout, (tpc, 1)), dtype=f),
    }
    return w


def _ensure_ntff_hook():
    """The RL container's antenv lacks axon_hooks; install a shim and
    register the ctypes NTFF profiler so trace=True works."""
    import types

    try:
        from antenv.axon_hooks import get_axon_ntff_profile_hook  # noqa
        return
    except ImportError:
        pass
    mod = types.ModuleType("antenv.axon_hooks")
    _h = [None]
    mod.set_axon_ntff_profile_hook = lambda h: _h.__setitem__(0, h)
    mod.get_axon_ntff_profile_hook = lambda: _h[0]
    sys.modules["antenv.axon_hooks"] = mod
    import antenv
    antenv.axon_hooks = mod
    try:
        from trn_agent_boot.trn_boot import _ntff_profile_via_ctypes
        h = _ntff_profile_via_ctypes("/opt/axon/libaxon_pjrt.so")
        if h is not None:
            mod.set_axon_ntff_profile_hook(h)
    except Exception:
        pass


def kernel(embeds, Wix, bix, Wih, Wfx, bfx, Wfh, Wox, box, Woh, Wux, bux, Wuh,
           Wout, bout, _trace=False):
    from concourse import bass_utils
    from concourse.bass_utils import run_bass_kernel_spmd

    if _trace:
        _ensure_ntff_hook()
        bass_utils.upload_artifacts = lambda d: d  # no S3 in this container

    n_cores = 8
    depth = 8
    T = 16
    tpc = T // n_cores

    embeds = np.asarray(embeds, dtype=np.float32)
    weights = _prep_weights(
        np.asarray(Wix), np.asarray(bix), np.asarray(Wih), np.asarray(Wfx),
        np.asarray(bfx), np.asarray(Wfh), np.asarray(Wox), np.asarray(box),
        np.asarray(Woh), np.asarray(Wux), np.asarray(bux), np.asarray(Wuh),
        np.asarray(Wout), np.asarray(bout), tpc)
    in_maps = _prep_core_inputs(embeds, weights, tpc, depth, n_cores)

    key = (tpc, depth)
    if key not in _NC_CACHE:
        _NC_CACHE[key] = build_nc(tpc=tpc, depth=depth)
    nc = _NC_CACHE[key]

    res = run_bass_kernel_spmd(nc, in_maps, core_ids=list(range(n_cores)),
                               trace=_trace)
    outs = np.concatenate([r["out"] for r in res.results], axis=0)
    if _trace:
        kernel.last_results = res
    return outs.astype(np.float32)


kernel.last_results = None
